# revision 1
# baseline (speedup 1.0000x reference)
"""Trainium2 Bass kernel for masked-softmax attention scoring.

Reference computation (B=128, T=512, K=1024, Q=1024):
    mids  = einsum("kq,bq->bk", W, query)
    s     = tanh(einsum("btk,bk->bt", key, mids) + bias)
    attn  = softmax-like: exp(s - max) * mask / sum(exp(s - max) * mask)

The max-subtraction cancels exactly in the ratio (tanh is bounded), so the
device computes  attn = exp(tanh(.)) * mask / sum_t(exp(tanh(.)) * mask).

Sharding: data-parallel over B across 8 NeuronCores (16 batches/core).
Per-core layout: partition p = (b, j) with b in [0,16), j in [0,8);
free column c in [0,64); timestep t = j*64 + c.

The mids matmul writes the (b, j)-replicated layout directly: the
stationary operand is query^T with each batch column replicated 8x via a
stride-0 DVE copy, fed as float32r (full-rate fp32 on the PE).  The W^T
prologue is split half-and-half across the two HWDGE FIFO rings so it
lands at aggregate HBM bandwidth; key chunks follow on both rings with
half-size chunks at the head and tail, consumed by 64 fused
multiply-reduce DVE ops (affine_mul_reduce) in merged arrival order.
Softmax normalization does the 8-partition group sum with a
block-diagonal 0/1 matmul.
"""

import sys

if "/opt/trn_rl_repo" not in sys.path:
    sys.path.insert(0, "/opt/trn_rl_repo")

from contextlib import ExitStack

import numpy as np

# ---- problem constants (hardcoded per spec) ----
B, T, K, Q = 128, 512, 1024, 1024
NCORES = 8
BS = B // NCORES          # 16 batches per core
P = 128                   # SBUF partitions
J = P // BS               # 8 t-blocks per batch on partitions
CF = T // J               # 64 timesteps per (partition, free col)
CC = 4                    # t-cols per key DMA super-chunk (2 MB each)
NCH = CF // CC            # 16 key DMAs per core
QC = Q // P               # 8 contraction chunks for the mids matmul
KEY_BUFS = 7              # key tile pool depth

_STATE: dict = {}


def _build_nc():
    import concourse.tile as tile
    from concourse import bacc, mybir

    f32 = mybir.dt.float32
    f32r = mybir.dt.float32r
    nc = bacc.Bacc()

    qt_e = nc.declare_dram_parameter("qt", [P, QC, BS], f32r, isOutput=False)
    wt_e = nc.declare_dram_parameter("wt", [P, QC, K], f32r, isOutput=False)
    grp_e = nc.declare_dram_parameter("grp", [P, P], f32, isOutput=False)
    key_e = nc.declare_dram_parameter("key", [BS, T, K], f32, isOutput=False)
    maskr_e = nc.declare_dram_parameter("maskr", [P, CF], f32, isOutput=False)
    bias_e = nc.declare_dram_parameter("biasb", [P, 1], f32, isOutput=False)
    out_e = nc.declare_dram_parameter("out", [P, CF], f32, isOutput=True)

    with tile.TileContext(nc) as tc, ExitStack() as ctx:
        const = ctx.enter_context(tc.tile_pool(name="const", bufs=1))
        kpool = ctx.enter_context(tc.tile_pool(name="key", bufs=KEY_BUFS))
        spool = ctx.enter_context(tc.tile_pool(name="scratch", bufs=2))
        psum = ctx.enter_context(tc.tile_pool(name="psum", bufs=1, space="PSUM"))

        # ---- prologue loads split across BOTH HWDGE rings (~2.1 MB each)
        # so W^T lands at full aggregate bandwidth (~20 us) and neither
        # ring idles before the key stream.
        qt_sb = const.tile([P, QC, BS], f32r)
        nc.sync.dma_start(out=qt_sb[:], in_=qt_e[:])
        wt_sb = const.tile([P, QC, K], f32r)
        for qc in range(QC // 2):
            nc.sync.dma_start(out=wt_sb[:, qc, :], in_=wt_e[:, qc, :])
        for qc in range(QC // 2, QC):
            nc.scalar.dma_start(out=wt_sb[:, qc, :], in_=wt_e[:, qc, :])
        grp_sb = const.tile([P, P], f32)
        nc.sync.dma_start(out=grp_sb[:], in_=grp_e[:])
        maskr_sb = const.tile([P, CF], f32)
        nc.sync.dma_start(out=maskr_sb[:], in_=maskr_e[:])
        bias_sb = const.tile([P, 1], f32)
        nc.sync.dma_start(out=bias_sb[:], in_=bias_e[:])

        # ---- mids in broadcast layout: [P, K], row p = mids[b(p), :] ----
        # Replicate each query column 8x on-chip (stride-0 DVE read) so the
        # stationary operand has the (b, j) partition order in one free dim.
        qtrep_sb = const.tile([P, QC, BS, J], f32r)
        nc.vector.tensor_copy(
            qtrep_sb[:], qt_sb[:].unsqueeze(-1).broadcast_to((P, QC, BS, J))
        )
        # matmuls in wt-chunk ARRIVAL order (rings deliver 0..3 and 4..7
        # concurrently); PSUM accumulation order is numerically immaterial.
        mids_ps = psum.tile([P, K], f32)
        qc_order = [0, 4, 1, 5, 2, 6, 3, 7]
        for qi, qc in enumerate(qc_order):
            lhsT = qtrep_sb[:, qc, :, :]
            for h in range(2):
                nc.tensor.matmul(
                    mids_ps[:, h * 512 : (h + 1) * 512],
                    lhsT=lhsT,
                    rhs=wt_sb[:, qc, h * 512 : (h + 1) * 512],
                    start=(qi == 0),
                    stop=(qi == QC - 1),
                )
        mids_bc = const.tile([P, K], f32)
        nc.vector.tensor_copy(mids_bc[:], mids_ps[:])

        # ---- scores[p, c] = key[b, j*64+c, :] . mids[b, :] ----
        # Both HWDGE FIFO rings stream 32 score-columns of key each, behind
        # their ~2.1 MB W^T halves; the final chunks are half-size so the
        # post-last-arrival DVE tail is short.  The DVE consumes chunks in
        # merged arrival order (model: equal per-ring column cadence).
        # Small chunks at the head (tolerate early arrival-order noise
        # cheaply) and at the tail (short post-last-arrival DVE tail).
        RING_COLS = {
            "A": [2, 2, 2, 2, 4, 4, 4, 4, 4, 2, 2],   # 32 cols
            "B": [4, 4, 4, 4, 4, 4, 4, 2, 2],          # 32 cols
        }
        entries = []
        for ring, pro in (("A", 6.8), ("B", 6.3)):
            t = pro
            for k, sz in enumerate(RING_COLS[ring]):
                t += sz * 2.9
                entries.append((t, ring, k, sz))
        entries.sort()
        scores_sb = const.tile([P, CF], f32)
        key_r = key_e[:].rearrange("b (j c) k -> (b j) c k", j=J)
        ring_eng = {"A": nc.sync, "B": nc.scalar}
        sched = []
        base = 0
        for t, ring, k, sz in entries:
            sched.append((ring, sz, base))
            base += sz
        for ring, sz, c0 in sched:
            kt = kpool.tile([P, CC, K], f32, tag="ktile")
            ring_eng[ring].dma_start(
                out=kt[:, 0:sz, :], in_=key_r[:, c0 : c0 + sz, :]
            )
            for cc in range(sz):
                c = c0 + cc
                prod = spool.tile([P, K], f32, tag="prod")
                nc.vector.affine_mul_reduce(
                    out=prod[:],
                    accum_out=scores_sb[:, c : c + 1],
                    in0=kt[:, cc, :],
                    in1=mids_bc[:],
                    scale=1.0,
                    bias=0.0,
                )

        # ---- epilogue: tanh, exp, mask, normalize ----
        tanh_sb = const.tile([P, CF], f32)
        nc.scalar.activation(
            out=tanh_sb[:],
            in_=scores_sb[:],
            func=mybir.ActivationFunctionType.Tanh,
            bias=bias_sb[:],
            scale=1.0,
        )
        exp_sb = const.tile([P, CF], f32)
        nc.scalar.activation(
            out=exp_sb[:], in_=tanh_sb[:], func=mybir.ActivationFunctionType.Exp
        )
        em_sb = const.tile([P, CF], f32)
        rowsum = const.tile([P, 1], f32)
        nc.vector.affine_mul_reduce(
            out=em_sb[:],
            accum_out=rowsum[:],
            in0=exp_sb[:],
            in1=maskr_sb[:],
            scale=1.0,
            bias=0.0,
        )
        den_ps = psum.tile([P, 1], f32)
        nc.tensor.matmul(
            den_ps[:], lhsT=grp_sb[:], rhs=rowsum[:], start=True, stop=True
        )
        rinv = const.tile([P, 1], f32)
        nc.vector.reciprocal(out=rinv[:], in_=den_ps[:])
        attn_sb = const.tile([P, CF], f32)
        nc.vector.tensor_scalar_mul(attn_sb[:], em_sb[:], rinv[:])
        nc.scalar.dma_start(out=out_e[:], in_=attn_sb[:])

    nc.compile()
    return nc


def _get_nc():
    if "nc" not in _STATE:
        _STATE["nc"] = _build_nc()
    return _STATE["nc"]


def _grp():
    if "GRP" not in _STATE:
        # GRP[p, m] = 1 iff p // J == m // J  (block-diagonal group-sum)
        pj = np.arange(P) // J
        _STATE["GRP"] = np.ascontiguousarray(
            (pj[:, None] == pj[None, :]).astype(np.float32)
        )
    return _STATE["GRP"]


def _make_in_maps(query, key, mask, W, bias):
    query = np.asarray(query, dtype=np.float32)
    key = np.asarray(key, dtype=np.float32)
    mask = np.asarray(mask, dtype=np.float32)
    W = np.asarray(W, dtype=np.float32)
    bias = np.asarray(bias, dtype=np.float32).reshape(-1)

    # wt[p, qc, k] = W.T[qc*128 + p, k]
    WT = np.ascontiguousarray(
        np.ascontiguousarray(W.T).reshape(QC, P, K).transpose(1, 0, 2)
    )
    GRP = _grp()
    biasb = np.ascontiguousarray(
        np.broadcast_to(bias[:1][None, :], (P, 1)).astype(np.float32)
    )

    in_maps = []
    for i in range(NCORES):
        sh = slice(i * BS, (i + 1) * BS)
        in_maps.append(
            {
                # pre-laid [P, QC, BS]: qt[p, qc, b] = query[sh].T[qc*128+p, b]
                "qt": np.ascontiguousarray(
                    query[sh].T.reshape(QC, P, BS).transpose(1, 0, 2)
                ),
                "wt": WT,
                "grp": GRP,
                "key": np.ascontiguousarray(key[sh]),
                "maskr": np.ascontiguousarray(mask[sh]).reshape(P, CF),
                "biasb": biasb,
            }
        )
    return in_maps


def _run(in_maps, **kwargs):
    from concourse.bass_utils import run_bass_kernel_spmd

    return run_bass_kernel_spmd(
        _get_nc(), in_maps, core_ids=list(range(NCORES)), **kwargs
    )


def _gather(results):
    return np.concatenate(
        [np.asarray(r["out"]).reshape(BS, T) for r in results], axis=0
    )


def kernel(query, key, mask, W, bias):
    in_maps = _make_in_maps(query, key, mask, W, bias)
    res = _run(in_maps)
    return _gather(res.results)



# revision 2
# speedup vs baseline: 1.4515x; 1.4515x over previous
"""Trainium2 Bass kernel for masked-softmax attention scoring.

Reference computation (B=128, T=512, K=1024, Q=1024):
    mids  = einsum("kq,bq->bk", W, query)
    s     = tanh(einsum("btk,bk->bt", key, mids) + bias)
    attn  = softmax-like: exp(s - max) * mask / sum(exp(s - max) * mask)

The max-subtraction cancels exactly in the ratio (tanh is bounded), so the
device computes  attn = exp(tanh(.)) * mask / sum_t(exp(tanh(.)) * mask).

Sharding: data-parallel over B across 8 NeuronCores (16 batches/core).

v2 design ("key through the weight port", fp16 wire format):
  * All large operands stream as fp16 (key 16 MB, W^T 2 MB per core), halving
    HBM traffic vs fp32.  Raw scores have std ~59 and tanh saturates to +-1,
    so fp16 rounding (~0.13 abs err on scores) perturbs only the ~3% of
    entries with |s| < 3, well inside the 2e-2 rel-l2 budget.
  * Host pre-transposes key to [kc, kp, b, t] so the contraction dim (k) is
    the SBUF partition dim.
  * mids^T = W @ query is computed directly in k-on-partitions layout:
    per (qc, kc) chunk, lhsT = W^T chunk [q, k] (128-col fp16 stationary,
    FWL), rhs = query^T chunk [q, b] -> accumulate mids_ps[k, (kc, b)].
  * scores: per (kc, b, tb) the key chunk [kp, 128 t] is the 128-col fp16
    stationary (fast weight load) and mids^T[:, kc, b] is a 1-column moving
    operand -> out [t(128 partitions), (tb, b)] accumulated over kc in ONE
    PSUM bank.  No diagonal extraction, trivial PSUM footprint, and the PE
    weight port does the key streaming at ~2 cols/cycle.
  * epilogue: tanh/exp on ScalarE straight out of PSUM, mask multiply on
    DVE, row sums (over the t partition dim) via a ones-vector matmul,
    reciprocal, partition-broadcast of 1/denom via a second ones matmul,
    final scale, DMA out.  PSUM accumulation uses a single start=True on the
    first matmul per bank (bank-fresh semantics make per-column groups safe).
"""

import sys

if "/opt/trn_rl_repo" not in sys.path:
    sys.path.insert(0, "/opt/trn_rl_repo")

from contextlib import ExitStack

import numpy as np

# ---- problem constants (hardcoded per spec) ----
B, T, K, Q = 128, 512, 1024, 1024
NCORES = 8
BS = B // NCORES          # 16 batches per core
P = 128                   # SBUF partitions
KC = K // P               # 8 contraction chunks for the scores matmuls
QC = Q // P               # 8 contraction chunks for the mids matmuls
TB = T // P               # 4 t-blocks of 128 (PSUM/output partition dim)
KEY_BUFS = 5              # key slab pool depth (16 KB/partition each)

_STATE: dict = {}


def _build_nc():
    import concourse.tile as tile
    from concourse import bacc, mybir

    f32 = mybir.dt.float32
    f16 = mybir.dt.float16
    nc = bacc.Bacc()

    keyt_e = nc.declare_dram_parameter("keyt", [KC, P, BS, T], f16, isOutput=False)
    wt_e = nc.declare_dram_parameter("wt", [P, QC, KC, P], f16, isOutput=False)
    qt_e = nc.declare_dram_parameter("qt", [P, QC, BS], f16, isOutput=False)
    maskr_e = nc.declare_dram_parameter("maskr", [P, TB, BS], f32, isOutput=False)
    bias_e = nc.declare_dram_parameter("biasb", [P, 1], f32, isOutput=False)
    out_e = nc.declare_dram_parameter("out", [P, TB, BS], f32, isOutput=True)

    with tile.TileContext(nc) as tc, ExitStack() as ctx:
        const = ctx.enter_context(tc.tile_pool(name="const", bufs=1))
        kpool = ctx.enter_context(tc.tile_pool(name="key", bufs=KEY_BUFS))
        psum = ctx.enter_context(tc.tile_pool(name="psum", bufs=1, space="PSUM"))

        # ---- small loads first, then W^T split across both HWDGE rings ----
        qt_sb = const.tile([P, QC, BS], f16)
        nc.sync.dma_start(out=qt_sb[:], in_=qt_e[:])
        mask_sb = const.tile([P, TB, BS], f32)
        nc.scalar.dma_start(out=mask_sb[:], in_=maskr_e[:])
        bias_sb = const.tile([P, 1], f32)
        nc.scalar.dma_start(out=bias_sb[:], in_=bias_e[:])

        wt_sb = const.tile([P, QC, KC, P], f16)
        for qc in range(QC // 2):
            nc.sync.dma_start(out=wt_sb[:, qc], in_=wt_e[:, qc])
        for qc in range(QC // 2, QC):
            nc.scalar.dma_start(out=wt_sb[:, qc], in_=wt_e[:, qc])

        ones_col = const.tile([P, 1], f32)
        nc.vector.memset(ones_col[:], 1.0)
        ones_row = const.tile([1, P], f32)
        nc.vector.memset(ones_row[:], 1.0)

        # ---- mids^T[k, (kc, b)] = sum_q W[k, q] query[b, q], qc chunks in
        # ring-arrival order (rings deliver qc 0-3 and 4-7 concurrently).
        # One start=True on the first matmul into the (fresh) bank.
        mids_ps = psum.tile([P, KC, BS], f32)
        qc_order = [0, 4, 1, 5, 2, 6, 3, 7]
        for qi, qc in enumerate(qc_order):
            for kc in range(KC):
                nc.tensor.matmul(
                    mids_ps[:, kc, :],
                    lhsT=wt_sb[:, qc, kc, :],
                    rhs=qt_sb[:, qc, :],
                    start=(qi == 0 and kc == 0),
                    stop=(qi == QC - 1),
                )
        mids_sb = const.tile([P, KC, BS], f16)
        nc.vector.tensor_copy(mids_sb[:], mids_ps[:])

        # ---- scores[t + 128 tb, (tb, b)] += key-chunk^T @ mids-col ----
        # Per kc slab (2 MB, half per ring): 64 stationary loads (fp16
        # 128-col -> fast weight load) each followed by a 1-column matmul.
        scores_ps = psum.tile([P, TB, BS], f32)
        for kc in range(KC):
            kt = kpool.tile([P, BS, T], f16, tag="ktile")
            nc.sync.dma_start(out=kt[:, 0 : BS // 2, :], in_=keyt_e[kc, :, 0 : BS // 2, :])
            nc.scalar.dma_start(
                out=kt[:, BS // 2 : BS, :], in_=keyt_e[kc, :, BS // 2 : BS, :]
            )
            for b in range(BS):
                for tb in range(TB):
                    nc.tensor.matmul(
                        scores_ps[:, tb, b : b + 1],
                        lhsT=kt[:, b, tb * P : (tb + 1) * P],
                        rhs=mids_sb[:, kc, b : b + 1],
                        start=(kc == 0 and b == 0 and tb == 0),
                        stop=(kc == KC - 1),
                    )

        # ---- epilogue: tanh, exp, mask, rowsum (ones matmul), normalize ----
        tanh_sb = const.tile([P, TB, BS], f32)
        nc.scalar.activation(
            out=tanh_sb[:],
            in_=scores_ps[:],
            func=mybir.ActivationFunctionType.Tanh,
            bias=bias_sb[:],
            scale=1.0,
        )
        exp_sb = const.tile([P, TB, BS], f32)
        nc.scalar.activation(
            out=exp_sb[:], in_=tanh_sb[:], func=mybir.ActivationFunctionType.Exp
        )
        em_sb = const.tile([P, TB, BS], f32)
        nc.vector.tensor_tensor(
            em_sb[:], exp_sb[:], mask_sb[:], mybir.AluOpType.mult
        )
        # sums over the t partition dim: [1, (tb, b)] = ones^T @ em
        sums_ps = psum.tile([1, TB, BS], f32)
        nc.tensor.matmul(
            sums_ps[:], lhsT=ones_col[:], rhs=em_sb[:], start=True, stop=True
        )
        sums_sb = const.tile([1, TB, BS], f32)
        nc.vector.tensor_copy(sums_sb[:], sums_ps[:])
        den_sb = const.tile([1, BS], f32)
        nc.vector.tensor_reduce(
            den_sb[:],
            sums_sb[:].rearrange("p tb b -> p b tb"),
            axis=mybir.AxisListType.X,
            op=mybir.AluOpType.add,
        )
        rden_sb = const.tile([1, BS], f32)
        nc.vector.reciprocal(out=rden_sb[:], in_=den_sb[:])
        # broadcast 1/denom across the 128 t partitions via a rank-1 matmul
        rden_ps = psum.tile([P, BS], f32)
        nc.tensor.matmul(
            rden_ps[:], lhsT=ones_row[:], rhs=rden_sb[:], start=True, stop=True
        )
        attn_sb = const.tile([P, TB, BS], f32)
        nc.vector.tensor_tensor(
            attn_sb[:],
            em_sb[:],
            rden_ps[:].unsqueeze(1).broadcast_to((P, TB, BS)),
            mybir.AluOpType.mult,
        )
        nc.sync.dma_start(out=out_e[:], in_=attn_sb[:])

    nc.compile()
    return nc


def _get_nc():
    if "nc" not in _STATE:
        _STATE["nc"] = _build_nc()
    return _STATE["nc"]


def _make_in_maps(query, key, mask, W, bias):
    query = np.asarray(query, dtype=np.float32)
    key = np.asarray(key, dtype=np.float32)
    mask = np.asarray(mask, dtype=np.float32)
    W = np.asarray(W, dtype=np.float32)
    bias = np.asarray(bias, dtype=np.float32).reshape(-1)

    # wt[qp, qc, kc, kl] = W[kc*128 + kl, qc*128 + qp]  (fp16)
    WT = np.ascontiguousarray(
        W.T.astype(np.float16)
        .reshape(QC, P, KC, P)
        .transpose(1, 0, 2, 3)
    )
    biasb = np.ascontiguousarray(
        np.broadcast_to(bias[:1][None, :], (P, 1)).astype(np.float32)
    )
    key16 = key.astype(np.float16)

    in_maps = []
    for i in range(NCORES):
        sh = slice(i * BS, (i + 1) * BS)
        # keyt[kc, kp, b, t] = key[b, t, kc*128 + kp]  (fp16)
        keyt = np.ascontiguousarray(key16[sh].transpose(2, 0, 1)).reshape(
            KC, P, BS, T
        )
        # qt[qp, qc, b] = query[b, qc*128 + qp]  (fp16)
        qt = np.ascontiguousarray(
            query[sh].T.astype(np.float16).reshape(QC, P, BS).transpose(1, 0, 2)
        )
        # maskr[tp, tb, b] = mask[b, tb*128 + tp]
        maskr = np.ascontiguousarray(
            mask[sh].T.reshape(TB, P, BS).transpose(1, 0, 2)
        )
        in_maps.append(
            {
                "keyt": keyt,
                "wt": WT,
                "qt": qt,
                "maskr": maskr,
                "biasb": biasb,
            }
        )
    return in_maps


def _run(in_maps, **kwargs):
    from concourse.bass_utils import run_bass_kernel_spmd

    return run_bass_kernel_spmd(
        _get_nc(), in_maps, core_ids=list(range(NCORES)), **kwargs
    )


def _gather(results):
    # out[tp, tb, b] -> attn[b, tb*128 + tp]
    return np.concatenate(
        [
            np.asarray(r["out"]).transpose(2, 1, 0).reshape(BS, T)
            for r in results
        ],
        axis=0,
    )


def kernel(query, key, mask, W, bias):
    in_maps = _make_in_maps(query, key, mask, W, bias)
    res = _run(in_maps)
    return _gather(res.results)


# revision 7
# speedup vs baseline: 1.6552x; 1.1404x over previous
"""Trainium2 Bass kernel for masked-softmax attention scoring.

Reference computation (B=128, T=512, K=1024, Q=1024):
    mids  = einsum("kq,bq->bk", W, query)
    s     = tanh(einsum("btk,bk->bt", key, mids) + bias)
    attn  = softmax-like: exp(s - max) * mask / sum(exp(s - max) * mask)

The max-subtraction cancels exactly in the ratio (tanh is bounded), so the
device computes  attn = exp(tanh(.)) * mask / sum_t(exp(tanh(.)) * mask).

Sharding: data-parallel over B across 8 NeuronCores (16 batches/core).

v4 design ("key through the weight port", mixed fp16/fp8 wire format):
  * Raw scores have std ~59 and tanh saturates hard, so precision on the
    k-contraction is cheap: the full-fp16 kernel measures rel-l2 1.4e-3
    against a 2e-2 budget.  v4 spends that margin on bandwidth: N_FP8 of the
    8 k-chunks of key stream as fp8-e4m3 (err std 4.2%/elem -> predicted
    rel-l2 ~1.4e-2 at N_FP8=2), cutting HBM traffic to 16.1 MB/core.
  * Host pre-transposes key to k-on-partitions layout; every dma_start reads
    a fully contiguous DRAM region (dense >=4 KB per-partition runs).
  * mids^T = W @ query computed directly in k-on-partitions layout:
    per (qc, kc) chunk, lhsT = W^T chunk [q, k] (128-col fp16 stationary,
    fast-weight-load), rhs = query^T chunk [q, b] -> mids_ps[k, (kc, b)].
  * scores: per (kc, b, tb) the key chunk [kp, 128 t] is the 128-col
    stationary (FWL, ~32 ns/load measured) and mids^T[:, kc, b] is a
    1-column moving operand -> out [t(128 partitions), (tb, b)] accumulated
    over kc in ONE PSUM bank.  No diagonal extraction, trivial PSUM
    footprint.  PE pair cost ~62 ns -> ~32 us, under the ~47 us DMA stream.
  * slab schedule: fp8 slabs (1 MB, 2.97 us) interleave between fp16 slabs
    (2 MB, 5.95 us) so slab arrival never outruns the PE's 4 us/slab; the
    last two fp16 slabs are quartered so the post-last-byte PE tail is ~1 us.
  * epilogue: tanh/exp straight out of PSUM on ScalarE, fp16 mask multiply
    on DVE, row sums over the t partition dim via 4 accumulating ones-vector
    matmuls, reciprocal, partition-broadcast of 1/denom via a rank-1 fp16
    ones matmul, final scale, DMA out.  PSUM accumulation uses a single
    start=True on the first matmul per bank (bank-fresh semantics make
    per-column accumulation groups safe).
"""

import sys

if "/opt/trn_rl_repo" not in sys.path:
    sys.path.insert(0, "/opt/trn_rl_repo")

from contextlib import ExitStack

import numpy as np

# ---- problem constants (hardcoded per spec) ----
B, T, K, Q = 128, 512, 1024, 1024
NCORES = 8
BS = B // NCORES          # 16 batches per core
P = 128                   # SBUF partitions
KC = K // P               # 8 contraction chunks for the scores matmuls
QC = Q // P               # 8 contraction chunks for the mids matmuls
TB = T // P               # 4 t-blocks of 128 (PSUM/output partition dim)
N_FP8 = 0                 # fp8-e4m3 key chunks: measured rel-l2 4.5e-2 at 2
                          # chunks (superlinear tanh-window sign flips) -- the
                          # 2e-2 gate forces all-fp16 key

N_F16 = KC - N_FP8
NQT = 2                   # last NQT fp16 slabs are quartered (short PE tail)
KEY_BUFS = 5              # fp16 slab pool depth (16 KB/partition each)

# slab issue order: logical kc by arrival position; fp8 slabs (N_FP8 of
# them, at the end of the logical range) interleave early/mid-stream, the
# quartered fp16 slabs go last.
_ORDER = [0, 6, 1, 2, 7, 3, 4, 5] if N_FP8 == 2 else (
    [0, 7, 1, 2, 3, 4, 5, 6] if N_FP8 == 1 else list(range(KC))
)

_STATE: dict = {}


def _build_nc():
    import concourse.tile as tile
    from concourse import bacc, mybir

    f32 = mybir.dt.float32
    f16 = mybir.dt.float16
    f8 = mybir.dt.float8e4
    nc = bacc.Bacc()

    # fp16 key chunks: first N_F16-NQT half-split, last NQT quarter-split
    kh_e = nc.declare_dram_parameter(
        "keyh", [N_F16 - NQT, 2, P, BS // 2, T], f16, isOutput=False
    )
    kq_e = nc.declare_dram_parameter(
        "keyq", [NQT, 4, P, BS // 4, T], f16, isOutput=False
    )
    k8_e = (
        nc.declare_dram_parameter("key8", [N_FP8, 2, P, BS // 2, T], f8, isOutput=False)
        if N_FP8
        else None
    )
    # wt[h, qp, qh, kc, kl] = W[kc*128 + kl, (h*4 + qh)*128 + qp]
    wt_e = nc.declare_dram_parameter(
        "wt", [2, P, QC // 2, KC, P], f16, isOutput=False
    )
    qt_e = nc.declare_dram_parameter("qt", [P, QC, BS], f16, isOutput=False)
    maskr_e = nc.declare_dram_parameter("maskr", [P, TB, BS], f16, isOutput=False)
    bias_e = nc.declare_dram_parameter("biasb", [P, 1], f32, isOutput=False)
    out_e = nc.declare_dram_parameter("out", [P, TB, BS], f32, isOutput=True)

    with tile.TileContext(nc) as tc, ExitStack() as ctx:
        const = ctx.enter_context(tc.tile_pool(name="const", bufs=1))
        kpool = ctx.enter_context(tc.tile_pool(name="key", bufs=KEY_BUFS))
        psum = ctx.enter_context(tc.tile_pool(name="psum", bufs=1, space="PSUM"))

        # ring A (sync) gets W half 0 immediately; tiny loads ride ring B.
        wt_sb = const.tile([P, 2, QC // 2, KC, P], f16)
        nc.sync.dma_start(out=wt_sb[:, 0], in_=wt_e[0])
        bias_sb = const.tile([P, 1], f32)
        nc.scalar.dma_start(out=bias_sb[:], in_=bias_e[:])
        qt_sb = const.tile([P, QC, BS], f16)
        nc.scalar.dma_start(out=qt_sb[:], in_=qt_e[:])
        nc.scalar.dma_start(out=wt_sb[:, 1], in_=wt_e[1])

        ones_col = const.tile([P, 1], f16)
        nc.vector.memset(ones_col[:], 1.0)
        ones_row = const.tile([1, P], f16)
        nc.vector.memset(ones_row[:], 1.0)

        # ---- mids^T[k, (kc, b)] = sum_q W[k, q] query[b, q], halves in
        # ring-arrival order.  Single start=True into the fresh bank.
        mids_ps = psum.tile([P, KC, BS], f32)
        for qi, (h, qh) in enumerate(
            [(0, 0), (1, 0), (0, 1), (1, 1), (0, 2), (1, 2), (0, 3), (1, 3)]
        ):
            for kc in range(KC):
                nc.tensor.matmul(
                    mids_ps[:, kc, :],
                    lhsT=wt_sb[:, h, qh, kc, :],
                    rhs=qt_sb[:, h * (QC // 2) + qh, :],
                    start=(qi == 0 and kc == 0),
                    stop=(qi == QC - 1),
                )
        mids_sb = const.tile([P, KC, BS], f16)
        nc.vector.tensor_copy(mids_sb[:], mids_ps[:])

        # mask arrives behind the second slab on ring B (epilogue-only use)
        mask_sb = const.tile([P, TB, BS], f16)

        # ---- scores[t + 128 tb, (tb, b)] += key-chunk^T @ mids-col ----
        scores_ps = psum.tile([P, TB, BS], f32)
        rings = [nc.sync, nc.scalar]
        nh = 0
        nq = 0
        n8 = 0
        for pos, kc in enumerate(_ORDER):
            if kc >= N_F16:
                kt = kpool.tile([P, BS, T], f8, tag="kt8")
                pieces = 2
                for pc in range(pieces):
                    w = BS // pieces
                    rings[pc % 2].dma_start(
                        out=kt[:, pc * w : (pc + 1) * w, :], in_=k8_e[n8, pc]
                    )
                n8 += 1
            elif kc >= N_F16 - NQT:
                kt = kpool.tile([P, BS, T], f16, tag="kt16")
                pieces = 4
                for pc in range(pieces):
                    w = BS // pieces
                    rings[pc % 2].dma_start(
                        out=kt[:, pc * w : (pc + 1) * w, :], in_=kq_e[nq, pc]
                    )
                nq += 1
            else:
                kt = kpool.tile([P, BS, T], f16, tag="kt16")
                pieces = 2
                for pc in range(pieces):
                    w = BS // pieces
                    rings[pc % 2].dma_start(
                        out=kt[:, pc * w : (pc + 1) * w, :], in_=kh_e[nh, pc]
                    )
                nh += 1
            if pos == 1:
                nc.scalar.dma_start(out=mask_sb[:], in_=maskr_e[:])
            for b in range(BS):
                for tb in range(TB):
                    nc.tensor.matmul(
                        scores_ps[:, tb, b : b + 1],
                        lhsT=kt[:, b, tb * P : (tb + 1) * P],
                        rhs=mids_sb[:, kc, b : b + 1],
                        start=(pos == 0 and b == 0 and tb == 0),
                        stop=(pos == KC - 1),
                    )

        # ---- epilogue: tanh, exp, mask, rowsum (ones matmuls), normalize ----
        tanh_sb = const.tile([P, TB, BS], f32)
        nc.scalar.activation(
            out=tanh_sb[:],
            in_=scores_ps[:],
            func=mybir.ActivationFunctionType.Tanh,
            bias=bias_sb[:],
            scale=1.0,
        )
        exp_sb = const.tile([P, TB, BS], f16)
        nc.scalar.activation(
            out=exp_sb[:], in_=tanh_sb[:], func=mybir.ActivationFunctionType.Exp
        )
        em_sb = const.tile([P, TB, BS], f16)
        nc.vector.tensor_tensor(
            em_sb[:], exp_sb[:], mask_sb[:], mybir.AluOpType.mult
        )
        sums_ps = psum.tile([1, BS], f32)
        for tb in range(TB):
            nc.tensor.matmul(
                sums_ps[:],
                lhsT=ones_col[:],
                rhs=em_sb[:, tb, :],
                start=(tb == 0),
                stop=(tb == TB - 1),
            )
        rden_sb = const.tile([1, BS], f16)
        with nc.allow_low_precision(reason="1/denom at fp16: rel 5e-4 << 2e-2"):
            nc.vector.reciprocal(out=rden_sb[:], in_=sums_ps[:])
        rden_ps = psum.tile([P, BS], f32)
        nc.tensor.matmul(
            rden_ps[:], lhsT=ones_row[:], rhs=rden_sb[:], start=True, stop=True
        )
        attn_sb = const.tile([P, TB, BS], f32)
        nc.vector.tensor_tensor(
            attn_sb[:],
            em_sb[:],
            rden_ps[:].unsqueeze(1).broadcast_to((P, TB, BS)),
            mybir.AluOpType.mult,
        )
        nc.sync.dma_start(out=out_e[:], in_=attn_sb[:])

    nc.compile()
    return nc


def _get_nc():
    if "nc" not in _STATE:
        _STATE["nc"] = _build_nc()
    return _STATE["nc"]


def _make_in_maps(query, key, mask, W, bias):
    from concourse import mybir

    f8np = mybir.dt.np(mybir.dt.float8e4)

    query = np.asarray(query, dtype=np.float32)
    key = np.asarray(key, dtype=np.float32)
    mask = np.asarray(mask, dtype=np.float32)
    W = np.asarray(W, dtype=np.float32)
    bias = np.asarray(bias, dtype=np.float32).reshape(-1)

    # wt[h, qp, qh, kc, kl] = W[kc*128 + kl, (h*4 + qh)*128 + qp]
    WT = np.ascontiguousarray(
        W.T.astype(np.float16)
        .reshape(2, QC // 2, P, KC, P)
        .transpose(0, 2, 1, 3, 4)
    )
    biasb = np.ascontiguousarray(
        np.broadcast_to(bias[:1][None, :], (P, 1)).astype(np.float32)
    )
    key16 = key.astype(np.float16)

    in_maps = []
    for i in range(NCORES):
        sh = slice(i * BS, (i + 1) * BS)
        # keyt[kc, kp, b, t] = key[b, t, kc*128 + kp]
        keyt = np.ascontiguousarray(key16[sh].transpose(2, 0, 1)).reshape(
            KC, P, BS, T
        )
        # fp16 half-split chunks: [n, 2, P, BS//2, T]
        keyh = np.ascontiguousarray(
            keyt[: N_F16 - NQT]
            .reshape(N_F16 - NQT, P, 2, BS // 2, T)
            .transpose(0, 2, 1, 3, 4)
        )
        # fp16 quarter-split chunks: [NQT, 4, P, BS//4, T]
        keyq = np.ascontiguousarray(
            keyt[N_F16 - NQT : N_F16]
            .reshape(NQT, P, 4, BS // 4, T)
            .transpose(0, 2, 1, 3, 4)
        )
        m = {
            "keyh": keyh,
            "keyq": keyq,
            "wt": WT,
            "qt": np.ascontiguousarray(
                query[sh].T.astype(np.float16).reshape(QC, P, BS).transpose(1, 0, 2)
            ),
            "maskr": np.ascontiguousarray(
                mask[sh].T.astype(np.float16).reshape(TB, P, BS).transpose(1, 0, 2)
            ),
            "biasb": biasb,
        }
        if N_FP8:
            m["key8"] = np.ascontiguousarray(
                keyt[N_F16:]
                .astype(f8np)
                .reshape(N_FP8, P, 2, BS // 2, T)
                .transpose(0, 2, 1, 3, 4)
            )
        in_maps.append(m)
    return in_maps


def _run(in_maps, **kwargs):
    from concourse.bass_utils import run_bass_kernel_spmd

    return run_bass_kernel_spmd(
        _get_nc(), in_maps, core_ids=list(range(NCORES)), **kwargs
    )


def _gather(results):
    # out[tp, tb, b] -> attn[b, tb*128 + tp]
    return np.concatenate(
        [
            np.asarray(r["out"]).transpose(2, 1, 0).reshape(BS, T)
            for r in results
        ],
        axis=0,
    )


def kernel(query, key, mask, W, bias):
    in_maps = _make_in_maps(query, key, mask, W, bias)
    res = _run(in_maps)
    return _gather(res.results)


# revision 12
# speedup vs baseline: 1.6748x; 1.0118x over previous
"""Trainium2 Bass kernel for masked-softmax attention scoring.

Reference computation (B=128, T=512, K=1024, Q=1024):
    mids  = einsum("kq,bq->bk", W, query)
    s     = tanh(einsum("btk,bk->bt", key, mids) + bias)
    attn  = softmax-like: exp(s - max) * mask / sum(exp(s - max) * mask)

The max-subtraction cancels exactly in the ratio (tanh is bounded), so the
device computes  attn = exp(tanh(.)) * mask / sum_t(exp(tanh(.)) * mask).

Sharding: data-parallel over B across 8 NeuronCores (16 batches/core).

v4 design ("key through the weight port", mixed fp16/fp8 wire format):
  * Raw scores have std ~59 and tanh saturates hard, so precision on the
    k-contraction is cheap: the full-fp16 kernel measures rel-l2 1.4e-3
    against a 2e-2 budget.  v4 spends that margin on bandwidth: N_FP8 of the
    8 k-chunks of key stream as fp8-e4m3 (err std 4.2%/elem -> predicted
    rel-l2 ~1.4e-2 at N_FP8=2), cutting HBM traffic to 16.1 MB/core.
  * Host pre-transposes key to k-on-partitions layout; every dma_start reads
    a fully contiguous DRAM region (dense >=4 KB per-partition runs).
  * mids^T = W @ query computed directly in k-on-partitions layout:
    per (qc, kc) chunk, lhsT = W^T chunk [q, k] (128-col fp16 stationary,
    fast-weight-load), rhs = query^T chunk [q, b] -> mids_ps[k, (kc, b)].
  * scores: per (kc, b, tb) the key chunk [kp, 128 t] is the 128-col
    stationary (FWL, ~32 ns/load measured) and mids^T[:, kc, b] is a
    1-column moving operand -> out [t(128 partitions), (tb, b)] accumulated
    over kc in ONE PSUM bank.  No diagonal extraction, trivial PSUM
    footprint.  PE pair cost ~62 ns -> ~32 us, under the ~47 us DMA stream.
  * slab schedule: fp8 slabs (1 MB, 2.97 us) interleave between fp16 slabs
    (2 MB, 5.95 us) so slab arrival never outruns the PE's 4 us/slab; the
    last two fp16 slabs are quartered so the post-last-byte PE tail is ~1 us.
  * epilogue: tanh/exp straight out of PSUM on ScalarE, fp16 mask multiply
    on DVE, row sums over the t partition dim via 4 accumulating ones-vector
    matmuls, reciprocal, partition-broadcast of 1/denom via a rank-1 fp16
    ones matmul, final scale, DMA out.  PSUM accumulation uses a single
    start=True on the first matmul per bank (bank-fresh semantics make
    per-column accumulation groups safe).
"""

import sys

if "/opt/trn_rl_repo" not in sys.path:
    sys.path.insert(0, "/opt/trn_rl_repo")

from contextlib import ExitStack

import numpy as np

# ---- problem constants (hardcoded per spec) ----
B, T, K, Q = 128, 512, 1024, 1024
NCORES = 8
BS = B // NCORES          # 16 batches per core
P = 128                   # SBUF partitions
KC = K // P               # 8 contraction chunks for the scores matmuls
QC = Q // P               # 8 contraction chunks for the mids matmuls
TB = T // P               # 4 t-blocks of 128 (PSUM/output partition dim)
N_FP8 = 0                 # fp8-e4m3 key chunks: measured rel-l2 4.5e-2 at 2
                          # chunks (superlinear tanh-window sign flips) -- the
                          # 2e-2 gate forces all-fp16 key

N_F16 = KC - N_FP8
NQT = 2                   # last NQT fp16 slabs are quartered (short PE tail)
KEY_BUFS = 5              # fp16 slab pool depth (16 KB/partition each)

# slab issue order: logical kc by arrival position; fp8 slabs (N_FP8 of
# them, at the end of the logical range) interleave early/mid-stream, the
# quartered fp16 slabs go last.
_ORDER = [0, 6, 1, 2, 7, 3, 4, 5] if N_FP8 == 2 else (
    [0, 7, 1, 2, 3, 4, 5, 6] if N_FP8 == 1 else list(range(KC))
)

_STATE: dict = {}


def _build_nc():
    import concourse.tile as tile
    from concourse import bacc, mybir

    f32 = mybir.dt.float32
    f16 = mybir.dt.float16
    f8 = mybir.dt.float8e4
    nc = bacc.Bacc()

    # fp16 key chunks: first N_F16-NQT half-split, last NQT quarter-split
    kh_e = nc.declare_dram_parameter(
        "keyh", [N_F16 - NQT, 2, P, BS // 2, T], f16, isOutput=False
    )
    kq_e = nc.declare_dram_parameter(
        "keyq", [NQT, 4, P, BS // 4, T], f16, isOutput=False
    )
    k8_e = (
        nc.declare_dram_parameter("key8", [N_FP8, 2, P, BS // 2, T], f8, isOutput=False)
        if N_FP8
        else None
    )
    # wt[h, qp, qh, kc, kl] = W[kc*128 + kl, (h*4 + qh)*128 + qp]
    wt_e = nc.declare_dram_parameter(
        "wt", [2, P, QC // 2, KC, P], f16, isOutput=False
    )
    qt_e = nc.declare_dram_parameter("qt", [P, QC, BS], f16, isOutput=False)
    maskr_e = nc.declare_dram_parameter("maskr", [P, TB, BS], f16, isOutput=False)
    bias_e = nc.declare_dram_parameter("biasb", [P, 1], f32, isOutput=False)
    out_e = nc.declare_dram_parameter("out", [P, TB, BS], f32, isOutput=True)

    with tile.TileContext(nc) as tc, ExitStack() as ctx:
        const = ctx.enter_context(tc.tile_pool(name="const", bufs=1))
        kpool = ctx.enter_context(tc.tile_pool(name="key", bufs=KEY_BUFS))
        qpool = ctx.enter_context(tc.tile_pool(name="keyq", bufs=2))
        psum = ctx.enter_context(tc.tile_pool(name="psum", bufs=1, space="PSUM"))

        # ring A (sync) gets W half 0 immediately; tiny loads ride ring B.
        # The halves are SEPARATE tiles: two dma_starts into one tile get
        # serialized by the framework's write-write dependency tracking
        # (v5 trace: ring B sat idle 9.4->12.2 us behind ring A's half).
        wt_sbs = [
            const.tile([P, QC // 2, KC, P], f16, tag=f"wt{h}", name=f"wt{h}")
            for h in range(2)
        ]
        nc.sync.dma_start(out=wt_sbs[0][:], in_=wt_e[0])
        bias_sb = const.tile([P, 1], f32)
        nc.scalar.dma_start(out=bias_sb[:], in_=bias_e[:])
        qt_sb = const.tile([P, QC, BS], f16)
        nc.scalar.dma_start(out=qt_sb[:], in_=qt_e[:])
        nc.scalar.dma_start(out=wt_sbs[1][:], in_=wt_e[1])

        ones_col = const.tile([P, 1], f16)
        nc.vector.memset(ones_col[:], 1.0)
        ones_row = const.tile([1, P], f16)
        nc.vector.memset(ones_row[:], 1.0)

        # ---- mids^T[k, (kc, b)] = sum_q W[k, q] query[b, q], halves in
        # ring-arrival order.  Single start=True into the fresh bank.
        mids_ps = psum.tile([P, KC, BS], f32)
        for qi, (h, qh) in enumerate(
            [(0, 0), (1, 0), (0, 1), (1, 1), (0, 2), (1, 2), (0, 3), (1, 3)]
        ):
            for kc in range(KC):
                nc.tensor.matmul(
                    mids_ps[:, kc, :],
                    lhsT=wt_sbs[h][:, qh, kc, :],
                    rhs=qt_sb[:, h * (QC // 2) + qh, :],
                    start=(qi == 0 and kc == 0),
                    stop=(qi == QC - 1),
                )
        mids_sb = const.tile([P, KC, BS], f16)
        nc.vector.tensor_copy(mids_sb[:], mids_ps[:])

        # mask arrives behind the second slab on ring B (epilogue-only use)
        mask_sb = const.tile([P, TB, BS], f16)

        # ---- scores[t + 128 tb, (tb, b)] += key-chunk^T @ mids-col ----
        # Every DMA piece gets its OWN tile so the two rings truly run in
        # parallel within a slab (shared-tile writes serialize).
        scores_ps = psum.tile([P, TB, BS], f32)
        rings = [nc.sync, nc.scalar]
        nh = 0
        nq = 0
        for pos, kc in enumerate(_ORDER):
            if kc >= N_F16 - NQT:
                pieces = 4
                w = BS // pieces
                tiles = []
                for pc in range(pieces):
                    t = qpool.tile([P, w, T], f16, tag=f"ktq{pc}", name=f"ktq{pc}")
                    rings[pc % 2].dma_start(out=t[:], in_=kq_e[nq, pc])
                    tiles.append(t)
                nq += 1
            else:
                pieces = 2
                w = BS // pieces
                tiles = []
                for pc in range(pieces):
                    t = kpool.tile([P, w, T], f16, tag=f"kth{pc}", name=f"kth{pc}")
                    rings[pc % 2].dma_start(out=t[:], in_=kh_e[nh, pc])
                    tiles.append(t)
                nh += 1
            if pos == 1:
                nc.scalar.dma_start(out=mask_sb[:], in_=maskr_e[:])
            for b in range(BS):
                kt = tiles[b // w]
                for tb in range(TB):
                    nc.tensor.matmul(
                        scores_ps[:, tb, b : b + 1],
                        lhsT=kt[:, b % w, tb * P : (tb + 1) * P],
                        rhs=mids_sb[:, kc, b : b + 1],
                        start=(pos == 0 and b == 0 and tb == 0),
                        stop=(pos == KC - 1),
                    )

        # ---- epilogue: tanh, exp, mask, rowsum (ones matmuls), normalize ----
        tanh_sb = const.tile([P, TB, BS], f32)
        nc.scalar.activation(
            out=tanh_sb[:],
            in_=scores_ps[:],
            func=mybir.ActivationFunctionType.Tanh,
            bias=bias_sb[:],
            scale=1.0,
        )
        exp_sb = const.tile([P, TB, BS], f16)
        nc.scalar.activation(
            out=exp_sb[:], in_=tanh_sb[:], func=mybir.ActivationFunctionType.Exp
        )
        em_sb = const.tile([P, TB, BS], f16)
        nc.vector.tensor_tensor(
            em_sb[:], exp_sb[:], mask_sb[:], mybir.AluOpType.mult
        )
        sums_ps = psum.tile([1, BS], f32)
        for tb in range(TB):
            nc.tensor.matmul(
                sums_ps[:],
                lhsT=ones_col[:],
                rhs=em_sb[:, tb, :],
                start=(tb == 0),
                stop=(tb == TB - 1),
            )
        rden_sb = const.tile([1, BS], f16)
        with nc.allow_low_precision(reason="1/denom at fp16: rel 5e-4 << 2e-2"):
            nc.vector.reciprocal(out=rden_sb[:], in_=sums_ps[:])
        rden_ps = psum.tile([P, BS], f32)
        nc.tensor.matmul(
            rden_ps[:], lhsT=ones_row[:], rhs=rden_sb[:], start=True, stop=True
        )
        attn_sb = const.tile([P, TB, BS], f32)
        nc.vector.tensor_tensor(
            attn_sb[:],
            em_sb[:],
            rden_ps[:].unsqueeze(1).broadcast_to((P, TB, BS)),
            mybir.AluOpType.mult,
        )
        nc.sync.dma_start(out=out_e[:], in_=attn_sb[:])

    nc.compile()
    return nc


def _get_nc():
    if "nc" not in _STATE:
        _STATE["nc"] = _build_nc()
    return _STATE["nc"]


def _make_in_maps(query, key, mask, W, bias):
    from concourse import mybir

    f8np = mybir.dt.np(mybir.dt.float8e4)

    query = np.asarray(query, dtype=np.float32)
    key = np.asarray(key, dtype=np.float32)
    mask = np.asarray(mask, dtype=np.float32)
    W = np.asarray(W, dtype=np.float32)
    bias = np.asarray(bias, dtype=np.float32).reshape(-1)

    # wt[h, qp, qh, kc, kl] = W[kc*128 + kl, (h*4 + qh)*128 + qp]
    WT = np.ascontiguousarray(
        W.T.astype(np.float16)
        .reshape(2, QC // 2, P, KC, P)
        .transpose(0, 2, 1, 3, 4)
    )
    biasb = np.ascontiguousarray(
        np.broadcast_to(bias[:1][None, :], (P, 1)).astype(np.float32)
    )
    key16 = key.astype(np.float16)

    in_maps = []
    for i in range(NCORES):
        sh = slice(i * BS, (i + 1) * BS)
        # keyt[kc, kp, b, t] = key[b, t, kc*128 + kp]
        keyt = np.ascontiguousarray(key16[sh].transpose(2, 0, 1)).reshape(
            KC, P, BS, T
        )
        # fp16 half-split chunks: [n, 2, P, BS//2, T]
        keyh = np.ascontiguousarray(
            keyt[: N_F16 - NQT]
            .reshape(N_F16 - NQT, P, 2, BS // 2, T)
            .transpose(0, 2, 1, 3, 4)
        )
        # fp16 quarter-split chunks: [NQT, 4, P, BS//4, T]
        keyq = np.ascontiguousarray(
            keyt[N_F16 - NQT : N_F16]
            .reshape(NQT, P, 4, BS // 4, T)
            .transpose(0, 2, 1, 3, 4)
        )
        m = {
            "keyh": keyh,
            "keyq": keyq,
            "wt": WT,
            "qt": np.ascontiguousarray(
                query[sh].T.astype(np.float16).reshape(QC, P, BS).transpose(1, 0, 2)
            ),
            "maskr": np.ascontiguousarray(
                mask[sh].T.astype(np.float16).reshape(TB, P, BS).transpose(1, 0, 2)
            ),
            "biasb": biasb,
        }
        if N_FP8:
            m["key8"] = np.ascontiguousarray(
                keyt[N_F16:]
                .astype(f8np)
                .reshape(N_FP8, P, 2, BS // 2, T)
                .transpose(0, 2, 1, 3, 4)
            )
        in_maps.append(m)
    return in_maps


def _run(in_maps, **kwargs):
    from concourse.bass_utils import run_bass_kernel_spmd

    return run_bass_kernel_spmd(
        _get_nc(), in_maps, core_ids=list(range(NCORES)), **kwargs
    )


def _gather(results):
    # out[tp, tb, b] -> attn[b, tb*128 + tp]
    return np.concatenate(
        [
            np.asarray(r["out"]).transpose(2, 1, 0).reshape(BS, T)
            for r in results
        ],
        axis=0,
    )


def kernel(query, key, mask, W, bias):
    in_maps = _make_in_maps(query, key, mask, W, bias)
    res = _run(in_maps)
    return _gather(res.results)


# revision 17
# speedup vs baseline: 1.7050x; 1.0180x over previous
"""Trainium2 Bass kernel for masked-softmax attention scoring.

Reference computation (B=128, T=512, K=1024, Q=1024):
    mids  = einsum("kq,bq->bk", W, query)
    s     = tanh(einsum("btk,bk->bt", key, mids) + bias)
    attn  = softmax-like: exp(s - max) * mask / sum(exp(s - max) * mask)

The max-subtraction cancels exactly in the ratio (tanh is bounded), so the
device computes  attn = exp(tanh(.)) * mask / sum_t(exp(tanh(.)) * mask).

Sharding: data-parallel over B across 8 NeuronCores (16 batches/core).

v4 design ("key through the weight port", mixed fp16/fp8 wire format):
  * Raw scores have std ~59 and tanh saturates hard, so precision on the
    k-contraction is cheap: the full-fp16 kernel measures rel-l2 1.4e-3
    against a 2e-2 budget.  v4 spends that margin on bandwidth: N_FP8 of the
    8 k-chunks of key stream as fp8-e4m3 (err std 4.2%/elem -> predicted
    rel-l2 ~1.4e-2 at N_FP8=2), cutting HBM traffic to 16.1 MB/core.
  * Host pre-transposes key to k-on-partitions layout; every dma_start reads
    a fully contiguous DRAM region (dense >=4 KB per-partition runs).
  * mids^T = W @ query computed directly in k-on-partitions layout:
    per (qc, kc) chunk, lhsT = W^T chunk [q, k] (128-col fp16 stationary,
    fast-weight-load), rhs = query^T chunk [q, b] -> mids_ps[k, (kc, b)].
  * scores: per (kc, b, tb) the key chunk [kp, 128 t] is the 128-col
    stationary (FWL, ~32 ns/load measured) and mids^T[:, kc, b] is a
    1-column moving operand -> out [t(128 partitions), (tb, b)] accumulated
    over kc in ONE PSUM bank.  No diagonal extraction, trivial PSUM
    footprint.  PE pair cost ~62 ns -> ~32 us, under the ~47 us DMA stream.
  * slab schedule: fp8 slabs (1 MB, 2.97 us) interleave between fp16 slabs
    (2 MB, 5.95 us) so slab arrival never outruns the PE's 4 us/slab; the
    last two fp16 slabs are quartered so the post-last-byte PE tail is ~1 us.
  * epilogue: tanh/exp straight out of PSUM on ScalarE, fp16 mask multiply
    on DVE, row sums over the t partition dim via 4 accumulating ones-vector
    matmuls, reciprocal, partition-broadcast of 1/denom via a rank-1 fp16
    ones matmul, final scale, DMA out.  PSUM accumulation uses a single
    start=True on the first matmul per bank (bank-fresh semantics make
    per-column accumulation groups safe).
"""

import sys

if "/opt/trn_rl_repo" not in sys.path:
    sys.path.insert(0, "/opt/trn_rl_repo")

from contextlib import ExitStack

import numpy as np

# ---- problem constants (hardcoded per spec) ----
B, T, K, Q = 128, 512, 1024, 1024
NCORES = 8
BS = B // NCORES          # 16 batches per core
P = 128                   # SBUF partitions
KC = K // P               # 8 contraction chunks for the scores matmuls
QC = Q // P               # 8 contraction chunks for the mids matmuls
TB = T // P               # 4 t-blocks of 128 (PSUM/output partition dim)
N_FP8 = 0                 # fp8-e4m3 key chunks: measured rel-l2 4.5e-2 at 2
                          # chunks (superlinear tanh-window sign flips) -- the
                          # 2e-2 gate forces all-fp16 key

N_F16 = KC - N_FP8
NQT = 2                   # last NQT fp16 slabs are quartered (short PE tail)
KEY_BUFS = 6              # fp16 half-slab pool depth per piece tag

# slab issue order: logical kc by arrival position; fp8 slabs (N_FP8 of
# them, at the end of the logical range) interleave early/mid-stream, the
# quartered fp16 slabs go last.
_ORDER = [0, 6, 1, 2, 7, 3, 4, 5] if N_FP8 == 2 else (
    [0, 7, 1, 2, 3, 4, 5, 6] if N_FP8 == 1 else list(range(KC))
)

_STATE: dict = {}


def _build_nc():
    import concourse.tile as tile
    from concourse import bacc, mybir

    f32 = mybir.dt.float32
    f16 = mybir.dt.float16
    f8 = mybir.dt.float8e4
    nc = bacc.Bacc()

    # fp16 key chunks: first N_F16-NQT half-split, last NQT quarter-split
    kh_e = nc.declare_dram_parameter(
        "keyh", [N_F16 - NQT, 2, P, BS // 2, T], f16, isOutput=False
    )
    kq_e = nc.declare_dram_parameter(
        "keyq", [NQT, 4, P, BS // 4, T], f16, isOutput=False
    )
    k8_e = (
        nc.declare_dram_parameter("key8", [N_FP8, 2, P, BS // 2, T], f8, isOutput=False)
        if N_FP8
        else None
    )
    # wt[h, qp, qh, kc, kl] = W[kc*128 + kl, (h*4 + qh)*128 + qp]
    wt_e = nc.declare_dram_parameter(
        "wt", [2, P, QC // 2, KC, P], f16, isOutput=False
    )
    qt_e = nc.declare_dram_parameter("qt", [P, QC, BS], f16, isOutput=False)
    maskr_e = nc.declare_dram_parameter("maskr", [P, TB, BS], f16, isOutput=False)
    bias_e = nc.declare_dram_parameter("biasb", [P, 1], f32, isOutput=False)
    out_e = nc.declare_dram_parameter("out", [P, TB, BS], f32, isOutput=True)

    with tile.TileContext(nc) as tc, ExitStack() as ctx:
        const = ctx.enter_context(tc.tile_pool(name="const", bufs=1))
        kpool = ctx.enter_context(tc.tile_pool(name="key", bufs=KEY_BUFS))
        qpool = ctx.enter_context(tc.tile_pool(name="keyq", bufs=2))
        psum = ctx.enter_context(tc.tile_pool(name="psum", bufs=1, space="PSUM"))

        # All dma_starts from both trigger engines land in the SAME 16 HW
        # queues in global enqueue (FIFO) order, and each dma_start costs
        # ~0.6-1 us of descriptor generation on its issuing sequencer.  So
        # enqueue order must match consumption order: both W halves first
        # (one per sequencer), then key slab pieces pairwise, tiny loads
        # late.
        wt_sbs = [
            const.tile([P, QC // 2, KC, P], f16, tag=f"wt{h}", name=f"wt{h}")
            for h in range(2)
        ]
        nc.sync.dma_start(out=wt_sbs[0][:], in_=wt_e[0])
        nc.scalar.dma_start(out=wt_sbs[1][:], in_=wt_e[1])
        bias_sb = const.tile([P, 1], f32)
        qt_sb = const.tile([P, QC, BS], f16)

        ones_col = const.tile([P, 1], f16)
        nc.vector.memset(ones_col[:], 1.0)
        ones_row = const.tile([1, P], f16)
        nc.vector.memset(ones_row[:], 1.0)

        # mask arrives late on ring B (epilogue-only use)
        mask_sb = const.tile([P, TB, BS], f16)
        rings = [nc.sync, nc.scalar]

        # Every DMA piece gets its own tile; pieces of one slab go out on
        # both trigger engines so their descriptors enqueue adjacently.
        _state = {"nh": 0, "nq": 0}

        def issue_slab_dmas(kc):
            if kc >= N_F16 - NQT:
                pieces, w = 4, BS // 4
                tiles = []
                for pc in range(pieces):
                    t = qpool.tile([P, w, T], f16, tag=f"ktq{pc}", name=f"ktq{pc}")
                    rings[pc % 2].dma_start(out=t[:], in_=kq_e[_state["nq"], pc])
                    tiles.append(t)
                _state["nq"] += 1
            else:
                pieces, w = 2, BS // 2
                tiles = []
                for pc in range(pieces):
                    t = kpool.tile([P, w, T], f16, tag=f"kth{pc}", name=f"kth{pc}")
                    rings[pc % 2].dma_start(out=t[:], in_=kh_e[_state["nh"], pc])
                    tiles.append(t)
                _state["nh"] += 1
            return tiles, w

        # slab 0's pieces and qt enqueue right behind the W halves, BEFORE
        # the mids matmuls are issued (their reads must follow the writes
        # in issue order)
        slab_q = [issue_slab_dmas(_ORDER[0])]
        nc.scalar.dma_start(out=qt_sb[:], in_=qt_e[:])

        # ---- mids^T[k, (kc, b)] = sum_q W[k, q] query[b, q] ----
        # Single start=True into the fresh bank.
        mids_ps = psum.tile([P, KC, BS], f32)
        for qi, (h, qh) in enumerate(
            [(0, 0), (1, 0), (0, 1), (1, 1), (0, 2), (1, 2), (0, 3), (1, 3)]
        ):
            for kc in range(KC):
                nc.tensor.matmul(
                    mids_ps[:, kc, :],
                    lhsT=wt_sbs[h][:, qh, kc, :],
                    rhs=qt_sb[:, h * (QC // 2) + qh, :],
                    start=(qi == 0 and kc == 0),
                    stop=(qi == QC - 1),
                )
        mids_sb = const.tile([P, KC, BS], f16)
        nc.vector.tensor_copy(mids_sb[:], mids_ps[:])

        # ---- scores[t + 128 tb, (tb, b)] += key-chunk^T @ mids-col ----
        scores_ps = psum.tile([P, TB, BS], f32)
        for pos, kc in enumerate(_ORDER):
            if pos + 1 < KC:
                slab_q.append(issue_slab_dmas(_ORDER[pos + 1]))
            tiles, w = slab_q[pos]
            for b in range(BS):
                kt = tiles[b // w]
                for tb in range(TB):
                    nc.tensor.matmul(
                        scores_ps[:, tb, b : b + 1],
                        lhsT=kt[:, b % w, tb * P : (tb + 1) * P],
                        rhs=mids_sb[:, kc, b : b + 1],
                        start=(pos == 0 and b == 0 and tb == 0),
                        stop=(pos == KC - 1),
                    )

        # epilogue-only loads: enqueued after every key piece, data lands
        # tens of us before the epilogue needs it
        nc.scalar.dma_start(out=bias_sb[:], in_=bias_e[:])
        nc.scalar.dma_start(out=mask_sb[:], in_=maskr_e[:])

        # ---- epilogue: tanh, exp, mask, rowsum (ones matmuls), normalize ----
        tanh_sb = const.tile([P, TB, BS], f32)
        nc.scalar.activation(
            out=tanh_sb[:],
            in_=scores_ps[:],
            func=mybir.ActivationFunctionType.Tanh,
            bias=bias_sb[:],
            scale=1.0,
        )
        exp_sb = const.tile([P, TB, BS], f16)
        nc.scalar.activation(
            out=exp_sb[:], in_=tanh_sb[:], func=mybir.ActivationFunctionType.Exp
        )
        em_sb = const.tile([P, TB, BS], f16)
        nc.vector.tensor_tensor(
            em_sb[:], exp_sb[:], mask_sb[:], mybir.AluOpType.mult
        )
        sums_ps = psum.tile([1, BS], f32)
        for tb in range(TB):
            nc.tensor.matmul(
                sums_ps[:],
                lhsT=ones_col[:],
                rhs=em_sb[:, tb, :],
                start=(tb == 0),
                stop=(tb == TB - 1),
            )
        rden_sb = const.tile([1, BS], f16)
        with nc.allow_low_precision(reason="1/denom at fp16: rel 5e-4 << 2e-2"):
            nc.vector.reciprocal(out=rden_sb[:], in_=sums_ps[:])
        rden_ps = psum.tile([P, BS], f32)
        nc.tensor.matmul(
            rden_ps[:], lhsT=ones_row[:], rhs=rden_sb[:], start=True, stop=True
        )
        attn_sb = const.tile([P, TB, BS], f32)
        nc.vector.tensor_tensor(
            attn_sb[:],
            em_sb[:],
            rden_ps[:].unsqueeze(1).broadcast_to((P, TB, BS)),
            mybir.AluOpType.mult,
        )
        nc.sync.dma_start(out=out_e[:], in_=attn_sb[:])

    nc.compile()
    return nc


def _get_nc():
    if "nc" not in _STATE:
        _STATE["nc"] = _build_nc()
    return _STATE["nc"]


def _make_in_maps(query, key, mask, W, bias):
    from concourse import mybir

    f8np = mybir.dt.np(mybir.dt.float8e4)

    query = np.asarray(query, dtype=np.float32)
    key = np.asarray(key, dtype=np.float32)
    mask = np.asarray(mask, dtype=np.float32)
    W = np.asarray(W, dtype=np.float32)
    bias = np.asarray(bias, dtype=np.float32).reshape(-1)

    # wt[h, qp, qh, kc, kl] = W[kc*128 + kl, (h*4 + qh)*128 + qp]
    WT = np.ascontiguousarray(
        W.T.astype(np.float16)
        .reshape(2, QC // 2, P, KC, P)
        .transpose(0, 2, 1, 3, 4)
    )
    biasb = np.ascontiguousarray(
        np.broadcast_to(bias[:1][None, :], (P, 1)).astype(np.float32)
    )
    key16 = key.astype(np.float16)

    in_maps = []
    for i in range(NCORES):
        sh = slice(i * BS, (i + 1) * BS)
        # keyt[kc, kp, b, t] = key[b, t, kc*128 + kp]
        keyt = np.ascontiguousarray(key16[sh].transpose(2, 0, 1)).reshape(
            KC, P, BS, T
        )
        # fp16 half-split chunks: [n, 2, P, BS//2, T]
        keyh = np.ascontiguousarray(
            keyt[: N_F16 - NQT]
            .reshape(N_F16 - NQT, P, 2, BS // 2, T)
            .transpose(0, 2, 1, 3, 4)
        )
        # fp16 quarter-split chunks: [NQT, 4, P, BS//4, T]
        keyq = np.ascontiguousarray(
            keyt[N_F16 - NQT : N_F16]
            .reshape(NQT, P, 4, BS // 4, T)
            .transpose(0, 2, 1, 3, 4)
        )
        m = {
            "keyh": keyh,
            "keyq": keyq,
            "wt": WT,
            "qt": np.ascontiguousarray(
                query[sh].T.astype(np.float16).reshape(QC, P, BS).transpose(1, 0, 2)
            ),
            "maskr": np.ascontiguousarray(
                mask[sh].T.astype(np.float16).reshape(TB, P, BS).transpose(1, 0, 2)
            ),
            "biasb": biasb,
        }
        if N_FP8:
            m["key8"] = np.ascontiguousarray(
                keyt[N_F16:]
                .astype(f8np)
                .reshape(N_FP8, P, 2, BS // 2, T)
                .transpose(0, 2, 1, 3, 4)
            )
        in_maps.append(m)
    return in_maps


def _run(in_maps, **kwargs):
    from concourse.bass_utils import run_bass_kernel_spmd

    return run_bass_kernel_spmd(
        _get_nc(), in_maps, core_ids=list(range(NCORES)), **kwargs
    )


def _gather(results):
    # out[tp, tb, b] -> attn[b, tb*128 + tp]
    return np.concatenate(
        [
            np.asarray(r["out"]).transpose(2, 1, 0).reshape(BS, T)
            for r in results
        ],
        axis=0,
    )


def kernel(query, key, mask, W, bias):
    in_maps = _make_in_maps(query, key, mask, W, bias)
    res = _run(in_maps)
    return _gather(res.results)


# revision 19
# speedup vs baseline: 1.7788x; 1.0433x over previous
"""Trainium2 Bass kernel for masked-softmax attention scoring.

Reference computation (B=128, T=512, K=1024, Q=1024):
    mids  = einsum("kq,bq->bk", W, query)
    s     = tanh(einsum("btk,bk->bt", key, mids) + bias)
    attn  = softmax-like: exp(s - max) * mask / sum(exp(s - max) * mask)

The max-subtraction cancels exactly in the ratio (tanh is bounded), so the
device computes  attn = exp(tanh(.)) * mask / sum_t(exp(tanh(.)) * mask).

Sharding: data-parallel over B across 8 NeuronCores (16 batches/core).

v4 design ("key through the weight port", mixed fp16/fp8 wire format):
  * Raw scores have std ~59 and tanh saturates hard, so precision on the
    k-contraction is cheap: the full-fp16 kernel measures rel-l2 1.4e-3
    against a 2e-2 budget.  v4 spends that margin on bandwidth: N_FP8 of the
    8 k-chunks of key stream as fp8-e4m3 (err std 4.2%/elem -> predicted
    rel-l2 ~1.4e-2 at N_FP8=2), cutting HBM traffic to 16.1 MB/core.
  * Host pre-transposes key to k-on-partitions layout; every dma_start reads
    a fully contiguous DRAM region (dense >=4 KB per-partition runs).
  * mids^T = W @ query computed directly in k-on-partitions layout:
    per (qc, kc) chunk, lhsT = W^T chunk [q, k] (128-col fp16 stationary,
    fast-weight-load), rhs = query^T chunk [q, b] -> mids_ps[k, (kc, b)].
  * scores: per (kc, b, tb) the key chunk [kp, 128 t] is the 128-col
    stationary (FWL, ~32 ns/load measured) and mids^T[:, kc, b] is a
    1-column moving operand -> out [t(128 partitions), (tb, b)] accumulated
    over kc in ONE PSUM bank.  No diagonal extraction, trivial PSUM
    footprint.  PE pair cost ~62 ns -> ~32 us, under the ~47 us DMA stream.
  * slab schedule: fp8 slabs (1 MB, 2.97 us) interleave between fp16 slabs
    (2 MB, 5.95 us) so slab arrival never outruns the PE's 4 us/slab; the
    last two fp16 slabs are quartered so the post-last-byte PE tail is ~1 us.
  * epilogue: tanh/exp straight out of PSUM on ScalarE, fp16 mask multiply
    on DVE, row sums over the t partition dim via 4 accumulating ones-vector
    matmuls, reciprocal, partition-broadcast of 1/denom via a rank-1 fp16
    ones matmul, final scale, DMA out.  PSUM accumulation uses a single
    start=True on the first matmul per bank (bank-fresh semantics make
    per-column accumulation groups safe).
"""

import sys

if "/opt/trn_rl_repo" not in sys.path:
    sys.path.insert(0, "/opt/trn_rl_repo")

from contextlib import ExitStack

import numpy as np

# ---- problem constants (hardcoded per spec) ----
B, T, K, Q = 128, 512, 1024, 1024
NCORES = 8
BS = B // NCORES          # 16 batches per core
P = 128                   # SBUF partitions
KC = K // P               # 8 contraction chunks for the scores matmuls
QC = Q // P               # 8 contraction chunks for the mids matmuls
TB = T // P               # 4 t-blocks of 128 (PSUM/output partition dim)
N_FP8 = 0                 # fp8-e4m3 key chunks: measured rel-l2 4.5e-2 at 2
                          # chunks (superlinear tanh-window sign flips) -- the
                          # 2e-2 gate forces all-fp16 key

N_F16 = KC - N_FP8
NQT = 2                   # last NQT fp16 slabs are quartered (short PE tail)
KEY_BUFS = 6              # fp16 half-slab pool depth per piece tag

# slab issue order: logical kc by arrival position; fp8 slabs (N_FP8 of
# them, at the end of the logical range) interleave early/mid-stream, the
# quartered fp16 slabs go last.
_ORDER = [0, 6, 1, 2, 7, 3, 4, 5] if N_FP8 == 2 else (
    [0, 7, 1, 2, 3, 4, 5, 6] if N_FP8 == 1 else list(range(KC))
)

_STATE: dict = {}


def _build_nc():
    import concourse.tile as tile
    from concourse import bacc, mybir

    f32 = mybir.dt.float32
    f16 = mybir.dt.float16
    f8 = mybir.dt.float8e4
    nc = bacc.Bacc()

    # fp16 key chunks: first N_F16-NQT half-split, last NQT quarter-split
    kh_e = nc.declare_dram_parameter(
        "keyh", [N_F16 - NQT, 2, P, BS // 2, T], f16, isOutput=False
    )
    kq_e = nc.declare_dram_parameter(
        "keyq", [NQT, 4, P, BS // 4, T], f16, isOutput=False
    )
    k8_e = (
        nc.declare_dram_parameter("key8", [N_FP8, 2, P, BS // 2, T], f8, isOutput=False)
        if N_FP8
        else None
    )
    # wt[h, qp, qh, kc, kl] = W[kc*128 + kl, (h*4 + qh)*128 + qp]
    wt_e = nc.declare_dram_parameter(
        "wt", [2, P, QC // 2, KC, P], f16, isOutput=False
    )
    qt_e = nc.declare_dram_parameter("qt", [P, QC, BS], f16, isOutput=False)
    maskr_e = nc.declare_dram_parameter("maskr", [P, TB, BS], f16, isOutput=False)
    bias_e = nc.declare_dram_parameter("biasb", [P, 1], f32, isOutput=False)
    out_e = nc.declare_dram_parameter("out", [P, TB, BS], f32, isOutput=True)

    with tile.TileContext(nc) as tc, ExitStack() as ctx:
        const = ctx.enter_context(tc.tile_pool(name="const", bufs=1))
        kpool = ctx.enter_context(tc.tile_pool(name="key", bufs=KEY_BUFS))
        qpool = ctx.enter_context(tc.tile_pool(name="keyq", bufs=2))
        psum = ctx.enter_context(tc.tile_pool(name="psum", bufs=1, space="PSUM"))

        # All dma_starts from both trigger engines land in the SAME 16 HW
        # queues in global enqueue (FIFO) order, and each dma_start costs
        # ~0.6-1 us of descriptor generation on its issuing sequencer.  So
        # enqueue order must match consumption order: both W halves first
        # (one per sequencer), then key slab pieces pairwise, tiny loads
        # late.
        wt_sbs = [
            const.tile([P, QC // 2, KC, P], f16, tag=f"wt{h}", name=f"wt{h}")
            for h in range(2)
        ]
        bias_sb = const.tile([P, 1], f32)
        qt_sb = const.tile([P, QC, BS], f16)
        nc.sync.dma_start(out=wt_sbs[0][:], in_=wt_e[0])
        nc.scalar.dma_start(out=qt_sb[:], in_=qt_e[:])
        nc.scalar.dma_start(out=wt_sbs[1][:], in_=wt_e[1])

        ones_col = const.tile([P, 1], f16)
        nc.vector.memset(ones_col[:], 1.0)
        ones_row = const.tile([1, P], f16)
        nc.vector.memset(ones_row[:], 1.0)

        # mask arrives late on ring B (epilogue-only use)
        mask_sb = const.tile([P, TB, BS], f16)
        rings = [nc.sync, nc.scalar]

        # Every DMA piece gets its own tile; pieces of one slab go out on
        # both trigger engines so their descriptors enqueue adjacently.
        _state = {"nh": 0, "nq": 0}

        def issue_slab_dmas(kc):
            if kc >= N_F16 - NQT:
                pieces, w = 4, BS // 4
                tiles = []
                for pc in range(pieces):
                    t = qpool.tile([P, w, T], f16, tag=f"ktq{pc}", name=f"ktq{pc}")
                    rings[pc % 2].dma_start(out=t[:], in_=kq_e[_state["nq"], pc])
                    tiles.append(t)
                _state["nq"] += 1
            else:
                pieces, w = 2, BS // 2
                tiles = []
                for pc in range(pieces):
                    t = kpool.tile([P, w, T], f16, tag=f"kth{pc}", name=f"kth{pc}")
                    rings[pc % 2].dma_start(out=t[:], in_=kh_e[_state["nh"], pc])
                    tiles.append(t)
                _state["nh"] += 1
            return tiles, w

        # slab 0's pieces enqueue right behind the W halves, BEFORE the
        # mids matmuls are issued (reads must follow writes in issue order)
        slab_q = [issue_slab_dmas(_ORDER[0])]

        # ---- mids^T[k, (kc, b)] = sum_q W[k, q] query[b, q] ----
        # h-major order: the wtA-half matmuls stream while wtB still loads.
        # Single start=True into the fresh bank.
        mids_ps = psum.tile([P, KC, BS], f32)
        for qi, (h, qh) in enumerate(
            [(0, 0), (0, 1), (0, 2), (0, 3), (1, 0), (1, 1), (1, 2), (1, 3)]
        ):
            for kc in range(KC):
                nc.tensor.matmul(
                    mids_ps[:, kc, :],
                    lhsT=wt_sbs[h][:, qh, kc, :],
                    rhs=qt_sb[:, h * (QC // 2) + qh, :],
                    start=(qi == 0 and kc == 0),
                    stop=(qi == QC - 1),
                )
        mids_sb = const.tile([P, KC, BS], f16)
        nc.vector.tensor_copy(mids_sb[:], mids_ps[:])

        # ---- scores[t + 128 tb, (tb, b)] += key-chunk^T @ mids-col ----
        scores_ps = psum.tile([P, TB, BS], f32)
        for pos, kc in enumerate(_ORDER):
            if pos + 1 < KC:
                slab_q.append(issue_slab_dmas(_ORDER[pos + 1]))
            tiles, w = slab_q[pos]
            for b in range(BS):
                kt = tiles[b // w]
                for tb in range(TB):
                    nc.tensor.matmul(
                        scores_ps[:, tb, b : b + 1],
                        lhsT=kt[:, b % w, tb * P : (tb + 1) * P],
                        rhs=mids_sb[:, kc, b : b + 1],
                        start=(pos == 0 and b == 0 and tb == 0),
                        stop=(pos == KC - 1),
                    )

        # epilogue-only loads: enqueued after every key piece, data lands
        # tens of us before the epilogue needs it
        nc.scalar.dma_start(out=bias_sb[:], in_=bias_e[:])
        nc.scalar.dma_start(out=mask_sb[:], in_=maskr_e[:])

        # ---- epilogue: tanh, exp, mask, rowsum (ones matmuls), normalize ----
        tanh_sb = const.tile([P, TB, BS], f32)
        nc.scalar.activation(
            out=tanh_sb[:],
            in_=scores_ps[:],
            func=mybir.ActivationFunctionType.Tanh,
            bias=bias_sb[:],
            scale=1.0,
        )
        exp_sb = const.tile([P, TB, BS], f16)
        nc.scalar.activation(
            out=exp_sb[:], in_=tanh_sb[:], func=mybir.ActivationFunctionType.Exp
        )
        em_sb = const.tile([P, TB, BS], f16)
        nc.vector.tensor_tensor(
            em_sb[:], exp_sb[:], mask_sb[:], mybir.AluOpType.mult
        )
        sums_ps = psum.tile([1, BS], f32)
        for tb in range(TB):
            nc.tensor.matmul(
                sums_ps[:],
                lhsT=ones_col[:],
                rhs=em_sb[:, tb, :],
                start=(tb == 0),
                stop=(tb == TB - 1),
            )
        rden_sb = const.tile([1, BS], f16)
        with nc.allow_low_precision(reason="1/denom at fp16: rel 5e-4 << 2e-2"):
            nc.vector.reciprocal(out=rden_sb[:], in_=sums_ps[:])
        rden_ps = psum.tile([P, BS], f32)
        nc.tensor.matmul(
            rden_ps[:], lhsT=ones_row[:], rhs=rden_sb[:], start=True, stop=True
        )
        attn_sb = const.tile([P, TB, BS], f32)
        nc.vector.tensor_tensor(
            attn_sb[:],
            em_sb[:],
            rden_ps[:].unsqueeze(1).broadcast_to((P, TB, BS)),
            mybir.AluOpType.mult,
        )
        nc.sync.dma_start(out=out_e[:], in_=attn_sb[:])

    nc.compile()
    return nc


def _get_nc():
    if "nc" not in _STATE:
        _STATE["nc"] = _build_nc()
    return _STATE["nc"]


def _make_in_maps(query, key, mask, W, bias):
    from concourse import mybir

    f8np = mybir.dt.np(mybir.dt.float8e4)

    query = np.asarray(query, dtype=np.float32)
    key = np.asarray(key, dtype=np.float32)
    mask = np.asarray(mask, dtype=np.float32)
    W = np.asarray(W, dtype=np.float32)
    bias = np.asarray(bias, dtype=np.float32).reshape(-1)

    # wt[h, qp, qh, kc, kl] = W[kc*128 + kl, (h*4 + qh)*128 + qp]
    WT = np.ascontiguousarray(
        W.T.astype(np.float16)
        .reshape(2, QC // 2, P, KC, P)
        .transpose(0, 2, 1, 3, 4)
    )
    biasb = np.ascontiguousarray(
        np.broadcast_to(bias[:1][None, :], (P, 1)).astype(np.float32)
    )
    key16 = key.astype(np.float16)

    in_maps = []
    for i in range(NCORES):
        sh = slice(i * BS, (i + 1) * BS)
        # keyt[kc, kp, b, t] = key[b, t, kc*128 + kp]
        keyt = np.ascontiguousarray(key16[sh].transpose(2, 0, 1)).reshape(
            KC, P, BS, T
        )
        # fp16 half-split chunks: [n, 2, P, BS//2, T]
        keyh = np.ascontiguousarray(
            keyt[: N_F16 - NQT]
            .reshape(N_F16 - NQT, P, 2, BS // 2, T)
            .transpose(0, 2, 1, 3, 4)
        )
        # fp16 quarter-split chunks: [NQT, 4, P, BS//4, T]
        keyq = np.ascontiguousarray(
            keyt[N_F16 - NQT : N_F16]
            .reshape(NQT, P, 4, BS // 4, T)
            .transpose(0, 2, 1, 3, 4)
        )
        m = {
            "keyh": keyh,
            "keyq": keyq,
            "wt": WT,
            "qt": np.ascontiguousarray(
                query[sh].T.astype(np.float16).reshape(QC, P, BS).transpose(1, 0, 2)
            ),
            "maskr": np.ascontiguousarray(
                mask[sh].T.astype(np.float16).reshape(TB, P, BS).transpose(1, 0, 2)
            ),
            "biasb": biasb,
        }
        if N_FP8:
            m["key8"] = np.ascontiguousarray(
                keyt[N_F16:]
                .astype(f8np)
                .reshape(N_FP8, P, 2, BS // 2, T)
                .transpose(0, 2, 1, 3, 4)
            )
        in_maps.append(m)
    return in_maps


def _run(in_maps, **kwargs):
    from concourse.bass_utils import run_bass_kernel_spmd

    return run_bass_kernel_spmd(
        _get_nc(), in_maps, core_ids=list(range(NCORES)), **kwargs
    )


def _gather(results):
    # out[tp, tb, b] -> attn[b, tb*128 + tp]
    return np.concatenate(
        [
            np.asarray(r["out"]).transpose(2, 1, 0).reshape(BS, T)
            for r in results
        ],
        axis=0,
    )


def kernel(query, key, mask, W, bias):
    in_maps = _make_in_maps(query, key, mask, W, bias)
    res = _run(in_maps)
    return _gather(res.results)
